# revision 11
# baseline (speedup 1.0000x reference)
"""Trainium2 Bass kernel for nn_ConvStackedTemporalGCN (8 NeuronCores, SPMD).

Strategy
--------
The reference network is, per period p:
    5 stacked GCNConv (linear, no activation between) -> H
    3 gate GCNConvs on Xp feeding a GRU-style cell    -> Hn
    H_accum += softmax(attn)[p] * Hn
then a node-local MLP readout.

Because gcn(h,W,b) = A(hW) + b with a FIXED normalized adjacency A, and A
(row mixing) commutes with W (column mixing), the 5 stacked convs collapse
to  H = (A^5 Xp) (W1..W5) + bias-terms, and the gate convs share Y1 = A Xp.
All 4 periods are packed into the feature dim (X [N, 4*128]), so the whole
message-passing phase is just FIVE sparse A-applications on [N, 512] data.

Sharding: nodes are partitioned contiguously across the 8 cores (1250
each, padded to 1280).  Each core computes A-rows for its own nodes
(dst-sorted edge chunks; gather via dma_gather + segment-sum via PE matmul
with per-chunk sparse selection matrices), then an AllGather replicates
the level output for the next hop's gathers.  The dense GRU/readout phase
is node-local and runs entirely on-core.
"""

import sys
import types

sys.path.insert(0, "/opt/trn_rl_repo")

import numpy as np

N, F, P, E = 10000, 128, 4, 160000
C, HID = 512, 256
CORES = 8
M = N // CORES          # 1250 real nodes per core
MP = 1280               # padded nodes per core
NP = CORES * MP         # 10240 padded global nodes
TILES = MP // 128       # 10 dst tiles per core
GSZ = 6                 # segment-sum chunks (of 128 edges) per dma_gather
WCOLS = GSZ * 128 // 16  # int16 index columns per gather

TRACE = False           # set kernel.TRACE=True before calling for profiling
LAST_RESULT = {}        # exec_time_ns etc. for test harness introspection

_CACHE = {}


def _install_ntff_hook():
    """trace=True under axon needs antenv.axon_hooks, absent in this image."""
    import antenv

    if "antenv.axon_hooks" in sys.modules:
        return
    mod = types.ModuleType("antenv.axon_hooks")
    _h = [None]
    mod.set_axon_ntff_profile_hook = lambda h: _h.__setitem__(0, h)
    mod.get_axon_ntff_profile_hook = lambda: _h[0]
    sys.modules["antenv.axon_hooks"] = mod
    antenv.axon_hooks = mod
    try:
        from trn_agent_boot.trn_boot import _ntff_profile_via_ctypes

        mod.set_axon_ntff_profile_hook(
            _ntff_profile_via_ctypes("/opt/axon/libaxon_pjrt.so")
        )
    except Exception:
        pass


def _build_program(KCH, NG, need_bias, probs, b2v):
    import concourse.bacc as bacc
    import concourse.mybir as mybir
    from concourse import tile
    from concourse.masks import make_identity
    from contextlib import ExitStack

    f32 = mybir.dt.float32
    i16 = mybir.dt.int16
    AF = mybir.ActivationFunctionType
    OP = mybir.AluOpType

    nc = bacc.Bacc(
        "TRN2",
        target_bir_lowering=False,
        debug=False,
        enable_asserts=True,
        num_devices=CORES,
    )
    xpk = nc.dram_tensor("xpk", [NP, C], f32, kind="ExternalInput")
    sblk = nc.dram_tensor("sblk", [128, TILES * KCH * 128], f32, kind="ExternalInput")
    gidx = nc.dram_tensor("gidx", [128, TILES * NG * WCOLS], i16, kind="ExternalInput")
    wcat = nc.dram_tensor("wcat", [128, 4 * C], f32, kind="ExternalInput")
    lcat = nc.dram_tensor("lcat", [128, 12 * C], f32, kind="ExternalInput")
    lw1 = nc.dram_tensor("lw1", [128, 4 * HID], f32, kind="ExternalInput")
    lw2 = nc.dram_tensor("lw2", [128, 2], f32, kind="ExternalInput")
    biasin = nc.dram_tensor("biasin", [128, 16], f32, kind="ExternalInput")
    out_h = nc.dram_tensor("out_h", [MP, C], f32, kind="ExternalOutput")
    out_y = nc.dram_tensor("out_y", [MP, 1], f32, kind="ExternalOutput")

    with tile.TileContext(nc) as tc, ExitStack() as ctx:
        dram = ctx.enter_context(tc.tile_pool(name="dram", bufs=1, space="DRAM"))
        yl = [
            dram.tile([MP, C], f32, name=f"yl{L}", tag=f"yl{L}") for L in range(5)
        ]
        yf = [
            dram.tile([NP, C], f32, name=f"yf{L}", tag=f"yf{L}", addr_space="Shared")
            for L in range(4)
        ]
        p0 = ctx.enter_context(tc.tile_pool(name="p0", bufs=1))
        gidx_t = p0.tile([128, TILES * NG * WCOLS], i16)
        ident = p0.tile([128, 128], f32)
        nc.sync.dma_start(gidx_t[:], gidx[:, :])
        make_identity(nc, ident[:])

        nreg = nc.gpsimd.to_reg(GSZ * 128)

        # ---------------- message passing: Y_{k+1} = A Y_k ----------------
        with tc.tile_pool(name="msg", bufs=1) as pm, tc.tile_pool(
            name="mpsum", bufs=2, space="PSUM"
        ) as mpsum:
            s_t = pm.tile([128, TILES * KCH * 128], f32, bufs=1)
            nc.sync.dma_start(s_t[:], sblk[:, :])
            for L in range(5):
                src = xpk if L == 0 else yf[L - 1]
                for t in range(TILES):
                    yp = mpsum.tile([128, C], f32, tag="ypsum", name="yp")
                    for g in range(NG):
                        G = pm.tile(
                            [128, GSZ * C], f32, tag="G", bufs=4, name="G"
                        )
                        cb = (t * NG + g) * WCOLS
                        nc.gpsimd.dma_gather(
                            out_ap=G[:].rearrange("p (c e) -> p c e", e=C),
                            in_ap=src[:, :],
                            idxs_ap=gidx_t[:, cb : cb + WCOLS],
                            num_idxs=GSZ * 128,
                            num_idxs_reg=nreg,
                            elem_size=C,
                        )
                        for k in range(GSZ):
                            ch = g * GSZ + k
                            scol = (t * KCH + ch) * 128
                            nc.tensor.matmul(
                                yp[:],
                                lhsT=s_t[:, scol : scol + 128],
                                rhs=G[:, k * C : (k + 1) * C],
                                start=(ch == 0),
                                stop=(ch == KCH - 1),
                            )
                    ysb = pm.tile([128, C], f32, tag="ysb", bufs=3, name="ysb")
                    nc.vector.tensor_copy(ysb[:], yp[:])
                    nc.sync.dma_start(yl[L][t * 128 : (t + 1) * 128, :], ysb[:])
                if L < 4:
                    nc.gpsimd.collective_compute(
                        "AllGather",
                        mybir.AluOpType.bypass,
                        replica_groups=[list(range(CORES))],
                        ins=[yl[L][:, :].opt()],
                        outs=[yf[L][:, :].opt()],
                    )

        # ---------------- dense node-local phase ----------------
        with tc.tile_pool(name="dn", bufs=1) as pd, tc.tile_pool(
            name="dpsum", bufs=2, space="PSUM"
        ) as dpsum:
            wcat_t = pd.tile([128, 4 * C], f32, bufs=1)
            lcat_t = pd.tile([128, 12 * C], f32, bufs=1)
            lw1_t = pd.tile([128, 4 * HID], f32, bufs=1)
            lw2_t = pd.tile([128, 2], f32, bufs=1)
            bias_t = pd.tile([128, 16], f32, bufs=1)
            nc.sync.dma_start(wcat_t[:], wcat[:, :])
            nc.sync.dma_start(lcat_t[:], lcat[:, :])
            nc.sync.dma_start(lw1_t[:], lw1[:, :])
            nc.sync.dma_start(lw2_t[:], lw2[:, :])
            nc.sync.dma_start(bias_t[:], biasin[:, :])

            for ng0, nt in [(0, 512), (512, 512), (1024, 256)]:
                nb = nt // 128
                # transpose Y1/Y5 node-rows into feature-major tiles
                yT = {}
                for nm, ylsrc in [("y1", yl[0]), ("y5", yl[4])]:
                    for k4 in range(4):
                        yT[(nm, k4)] = pd.tile(
                            [128, nt], f32, tag=f"{nm}T{k4}", bufs=1, name="yTt"
                        )
                    for b in range(nb):
                        rt = pd.tile([128, C], f32, tag="rt", bufs=3, name="rt")
                        nc.sync.dma_start(
                            rt[:], ylsrc[ng0 + b * 128 : ng0 + (b + 1) * 128, :]
                        )
                        for k4 in range(4):
                            tp = dpsum.tile(
                                [128, 128], f32, tag="tp", name="tp"
                            )
                            nc.tensor.transpose(
                                tp[:], rt[:, k4 * 128 : (k4 + 1) * 128], ident[:]
                            )
                            nc.vector.tensor_copy(
                                yT[(nm, k4)][:, b * 128 : (b + 1) * 128], tp[:]
                            )
                accs = [None] * 4
                for p in range(P):
                    H = [
                        pd.tile([128, nt], f32, tag=f"H{m}", bufs=2, name="Ht_")
                        for m in range(4)
                    ]
                    for m in range(4):
                        hp = dpsum.tile([128, nt], f32, tag="dp", name="hp")
                        nc.tensor.matmul(
                            hp[:],
                            lhsT=wcat_t[:, m * 128 : (m + 1) * 128],
                            rhs=yT[("y5", p)][:],
                            start=True,
                            stop=True,
                        )
                        nc.vector.tensor_copy(H[m][:], hp[:])
                    gates = []
                    for gi, func in [(1, AF.Sigmoid), (2, AF.Sigmoid)]:
                        gt = [
                            pd.tile(
                                [128, nt], f32, tag=f"g{gi}{m}", bufs=1, name="gt"
                            )
                            for m in range(4)
                        ]
                        for m in range(4):
                            zp = dpsum.tile([128, nt], f32, tag="dp", name="zp")
                            nc.tensor.matmul(
                                zp[:],
                                lhsT=wcat_t[:, gi * C + m * 128 : gi * C + (m + 1) * 128],
                                rhs=yT[("y1", p)][:],
                                start=True,
                                stop=False,
                            )
                            Lsec = (gi - 1) * 4 * C
                            for k in range(4):
                                nc.tensor.matmul(
                                    zp[:],
                                    lhsT=lcat_t[
                                        :,
                                        Lsec + k * C + m * 128 : Lsec + k * C + (m + 1) * 128,
                                    ],
                                    rhs=H[k][:],
                                    start=False,
                                    stop=(k == 3),
                                )
                            bcol = (gi - 1) * 4 + m
                            if need_bias:
                                nc.scalar.activation(
                                    gt[m][:], zp[:], func,
                                    bias=bias_t[:, bcol : bcol + 1],
                                )
                            else:
                                nc.scalar.activation(gt[m][:], zp[:], func)
                        gates.append(gt)
                    Z, Rg = gates
                    HR = [
                        pd.tile([128, nt], f32, tag=f"HR{m}", bufs=1, name="HRt")
                        for m in range(4)
                    ]
                    for m in range(4):
                        nc.vector.tensor_tensor(
                            out=HR[m][:], in0=H[m][:], in1=Rg[m][:], op=OP.mult
                        )
                    newacc = [None] * 4
                    for m in range(4):
                        tp2 = dpsum.tile([128, nt], f32, tag="dp", name="tp2")
                        nc.tensor.matmul(
                            tp2[:],
                            lhsT=wcat_t[:, 3 * C + m * 128 : 3 * C + (m + 1) * 128],
                            rhs=yT[("y1", p)][:],
                            start=True,
                            stop=False,
                        )
                        for k in range(4):
                            nc.tensor.matmul(
                                tp2[:],
                                lhsT=lcat_t[
                                    :,
                                    8 * C + k * C + m * 128 : 8 * C + k * C + (m + 1) * 128,
                                ],
                                rhs=HR[k][:],
                                start=False,
                                stop=(k == 3),
                            )
                        Htn = pd.tile([128, nt], f32, tag="Htn", bufs=2, name="Htn")
                        if need_bias:
                            nc.scalar.activation(
                                Htn[:], tp2[:], AF.Tanh, bias=bias_t[:, 8 + m : 9 + m]
                            )
                        else:
                            nc.scalar.activation(Htn[:], tp2[:], AF.Tanh)
                        d1 = pd.tile([128, nt], f32, tag="d1", bufs=2, name="d1")
                        nc.vector.tensor_tensor(
                            out=d1[:], in0=H[m][:], in1=Htn[:], op=OP.subtract
                        )
                        d2 = pd.tile([128, nt], f32, tag="d2", bufs=2, name="d2")
                        nc.vector.tensor_tensor(
                            out=d2[:], in0=Z[m][:], in1=d1[:], op=OP.mult
                        )
                        hn = pd.tile([128, nt], f32, tag="hn", bufs=2, name="hn")
                        nc.vector.tensor_tensor(
                            out=hn[:], in0=d2[:], in1=Htn[:], op=OP.add
                        )
                        na = pd.tile([128, nt], f32, tag=f"acc{m}", bufs=2, name="na")
                        if p == 0:
                            nc.vector.tensor_scalar_mul(na[:], hn[:], float(probs[0]))
                        else:
                            nc.vector.scalar_tensor_tensor(
                                out=na[:], in0=hn[:], scalar=float(probs[p]),
                                in1=accs[m][:], op0=OP.mult, op1=OP.add,
                            )
                        newacc[m] = na
                    accs = newacc
                # readout
                rl = [
                    pd.tile([128, nt], f32, tag=f"rl{m}", bufs=1, name="rl")
                    for m in range(4)
                ]
                for m in range(4):
                    nc.scalar.activation(rl[m][:], accs[m][:], AF.Relu)
                h1 = []
                for hm in range(2):
                    pp = dpsum.tile([128, nt], f32, tag="dp", name="pp")
                    for k in range(4):
                        nc.tensor.matmul(
                            pp[:],
                            lhsT=lw1_t[:, k * HID + hm * 128 : k * HID + (hm + 1) * 128],
                            rhs=rl[k][:],
                            start=(k == 0),
                            stop=(k == 3),
                        )
                    t1 = pd.tile([128, nt], f32, tag=f"h1{hm}", bufs=1, name="t1")
                    if need_bias:
                        nc.scalar.activation(
                            t1[:], pp[:], AF.Relu, bias=bias_t[:, 12 + hm : 13 + hm]
                        )
                    else:
                        nc.scalar.activation(t1[:], pp[:], AF.Relu)
                    h1.append(t1)
                py = dpsum.tile([1, nt], f32, tag="py", name="py")
                for hm in range(2):
                    nc.tensor.matmul(
                        py[:],
                        lhsT=lw2_t[:, hm : hm + 1],
                        rhs=h1[hm][:],
                        start=(hm == 0),
                        stop=(hm == 1),
                    )
                oy = pd.tile([1, nt], f32, tag="oy", bufs=2, name="oy")
                nc.vector.tensor_scalar_add(oy[:], py[:], float(b2v))
                nc.sync.dma_start(
                    out_y[ng0 : ng0 + nt, 0:1].rearrange("a b -> b a"),
                    oy[0:1, :nt],
                )
                for m in range(4):
                    for b in range(nb):
                        tp = dpsum.tile([128, 128], f32, tag="tp", name="tp3")
                        nc.tensor.transpose(
                            tp[:], accs[m][:, b * 128 : (b + 1) * 128], ident[:]
                        )
                        hsb = pd.tile([128, 128], f32, tag="hsb", bufs=3, name="hsb")
                        nc.vector.tensor_copy(hsb[:], tp[:])
                        nc.sync.dma_start(
                            out_h[
                                ng0 + b * 128 : ng0 + (b + 1) * 128,
                                m * 128 : (m + 1) * 128,
                            ],
                            hsb[:],
                        )
    nc.compile()
    return nc


def kernel(**inputs):
    x = np.asarray(inputs["x"], np.float32)
    edge_index = np.asarray(inputs["edge_index"])
    edge_attr = np.asarray(inputs["edge_attr"], np.float32)

    # ---- graph preprocessing (host): norm, partition, dst-sorted chunks ----
    src = np.concatenate([edge_index[0], np.arange(N)]).astype(np.int64)
    dst = np.concatenate([edge_index[1], np.arange(N)]).astype(np.int64)
    ew = np.concatenate([edge_attr, np.ones(N, np.float32)]).astype(np.float32)
    deg = np.zeros(N, np.float32)
    np.add.at(deg, dst, ew)
    dinv = np.where(deg > 0, 1.0 / np.sqrt(np.where(deg > 0, deg, 1.0)), 0.0).astype(
        np.float32
    )
    norm = (dinv[src] * ew * dinv[dst]).astype(np.float32)
    core_of = dst // M
    dst_local = dst % M
    src_pad = ((src // M) * MP + (src % M)).astype(np.int64)

    # per (core, tile) edge lists
    per_ct = [[None] * TILES for _ in range(CORES)]
    counts = np.zeros((CORES, TILES), np.int64)
    tile_of = dst_local // 128
    key = core_of * TILES + tile_of
    order = np.argsort(key * (M + 1) + dst_local, kind="stable")
    s_src, s_norm, s_dl, s_key = (
        src_pad[order],
        norm[order],
        dst_local[order],
        key[order],
    )
    bounds = np.searchsorted(s_key, np.arange(CORES * TILES + 1))
    for c in range(CORES):
        for t in range(TILES):
            a, b = bounds[c * TILES + t], bounds[c * TILES + t + 1]
            per_ct[c][t] = (s_src[a:b], s_norm[a:b], s_dl[a:b] - t * 128)
            counts[c, t] = b - a
    KCH_need = int(np.max((counts + 127) // 128))
    NG = (KCH_need + GSZ - 1) // GSZ
    KCH = NG * GSZ

    sblks = []
    gidxs = []
    for c in range(CORES):
        S = np.zeros((128, TILES * KCH, 128), np.float32)
        IDX = np.zeros((TILES * KCH * 128,), np.int16)
        for t in range(TILES):
            e_src, e_norm, e_d = per_ct[c][t]
            n = len(e_src)
            base = t * KCH * 128
            sl = np.arange(n)
            S[sl % 128, t * KCH + sl // 128, e_d] = e_norm
            IDX[base : base + n] = e_src.astype(np.int16)
        # wrap indices per gather: [16, WCOLS] blocks replicated to 128 parts
        gi = np.zeros((128, TILES * NG * WCOLS), np.int16)
        for t in range(TILES):
            for g in range(NG):
                a = t * KCH * 128 + g * GSZ * 128
                arr = IDX[a : a + GSZ * 128]
                blk = arr.reshape(WCOLS, 16).T  # [16, WCOLS]
                gi[:, (t * NG + g) * WCOLS : (t * NG + g + 1) * WCOLS] = np.tile(
                    blk, (8, 1)
                )
        sblks.append(S.reshape(128, TILES * KCH * 128))
        gidxs.append(gi)

    # ---- weight composition (host, fp64 -> fp32) ----
    W = [np.asarray(inputs[f"W{i}"], np.float64) for i in range(1, 6)]
    bvec = [np.asarray(inputs[f"b{i}"], np.float64) for i in range(1, 6)]
    Lz = np.asarray(inputs["Lz_w"], np.float64)
    Lr = np.asarray(inputs["Lr_w"], np.float64)
    Lh = np.asarray(inputs["Lh_w"], np.float64)
    Wc = W[0]
    for Wi in W[1:]:
        Wc = Wc @ Wi
    Wzp = np.asarray(inputs["Wz"], np.float64) @ Lz[:C]
    Wrp = np.asarray(inputs["Wr"], np.float64) @ Lr[:C]
    Whp = np.asarray(inputs["Wh"], np.float64) @ Lh[:C]
    wcat = np.concatenate([Wc, Wzp, Wrp, Whp], axis=1).astype(np.float32)

    def chunkrows(A):  # [512, X] -> [128, 4*X]
        return np.concatenate([A[k * 128 : (k + 1) * 128] for k in range(4)], 1)

    lcat = np.concatenate(
        [
            chunkrows(Lz[C:].astype(np.float32)),
            chunkrows(Lr[C:].astype(np.float32)),
            chunkrows(Lh[C:].astype(np.float32)),
        ],
        axis=1,
    )
    lw1 = chunkrows(np.asarray(inputs["lin1_w"], np.float32))
    lw2m = np.asarray(inputs["lin2_w"], np.float32)  # [256, 1]
    lw2 = np.stack([lw2m[:128, 0], lw2m[128:, 0]], axis=1)  # [128, 2]

    bz = np.asarray(inputs["bz"], np.float64) @ Lz[:C] + np.asarray(
        inputs["Lz_b"], np.float64
    )
    br = np.asarray(inputs["br"], np.float64) @ Lr[:C] + np.asarray(
        inputs["Lr_b"], np.float64
    )
    bh = np.asarray(inputs["bh"], np.float64) @ Lh[:C] + np.asarray(
        inputs["Lh_b"], np.float64
    )
    lin1_b = np.asarray(inputs["lin1_b"], np.float64)
    biasarr = np.zeros((128, 16), np.float32)
    for mm in range(4):
        biasarr[:, mm] = bz[mm * 128 : (mm + 1) * 128]
        biasarr[:, 4 + mm] = br[mm * 128 : (mm + 1) * 128]
        biasarr[:, 8 + mm] = bh[mm * 128 : (mm + 1) * 128]
    biasarr[:, 12] = lin1_b[:128]
    biasarr[:, 13] = lin1_b[128:]
    need_bias = bool(np.abs(biasarr).max() > 0)
    # stacked-conv bias correction must be zero for the composed-weight path
    assert all(np.abs(b).max() == 0 for b in bvec), "nonzero conv bias unsupported"

    attn = np.asarray(inputs["attn"], np.float64)
    probs = np.exp(attn - attn.max())
    probs = (probs / probs.sum()).astype(np.float32)
    b2v = float(np.asarray(inputs["lin2_b"])[0])

    # ---- packed node features, padded layout ----
    xpk = np.zeros((NP, C), np.float32)
    xr = x.transpose(0, 2, 1).reshape(N, P * F)
    for c in range(CORES):
        xpk[c * MP : c * MP + M] = xr[c * M : (c + 1) * M]

    # ---- build / fetch program ----
    key2 = (KCH, NG, need_bias, tuple(np.round(probs, 7)), round(b2v, 7))
    if key2 not in _CACHE:
        _CACHE[key2] = _build_program(KCH, NG, need_bias, probs, b2v)
    nc = _CACHE[key2]

    from concourse.bass_utils import run_bass_kernel_spmd

    in_maps = []
    for c in range(CORES):
        in_maps.append(
            {
                "xpk": xpk,
                "sblk": sblks[c],
                "gidx": gidxs[c],
                "wcat": wcat,
                "lcat": lcat,
                "lw1": lw1,
                "lw2": lw2,
                "biasin": biasarr,
            }
        )
    if TRACE:
        _install_ntff_hook()
    res = run_bass_kernel_spmd(
        nc, in_maps, core_ids=list(range(CORES)), trace=TRACE
    )
    LAST_RESULT["exec_time_ns"] = res.exec_time_ns

    out_hidden = np.concatenate(
        [res.results[c]["out_h"][:M] for c in range(CORES)], axis=0
    )
    out = np.concatenate([res.results[c]["out_y"][:M] for c in range(CORES)], axis=0)
    return out, out_hidden


# revision 13
# speedup vs baseline: 1.0739x; 1.0739x over previous
"""Trainium2 Bass kernel for nn_ConvStackedTemporalGCN (8 NeuronCores, SPMD).

Strategy
--------
The reference network is, per period p:
    5 stacked GCNConv (linear, no activation between) -> H
    3 gate GCNConvs on Xp feeding a GRU-style cell    -> Hn
    H_accum += softmax(attn)[p] * Hn
then a node-local MLP readout.

Because gcn(h,W,b) = A(hW) + b with a FIXED normalized adjacency A, and A
(row mixing) commutes with W (column mixing), the 5 stacked convs collapse
to  H = (A^5 Xp) (W1..W5) + bias-terms, and the gate convs share Y1 = A Xp.
All 4 periods are packed into the feature dim (X [N, 4*128]), so the whole
message-passing phase is just FIVE sparse A-applications on [N, 512] data.

Sharding: nodes are partitioned contiguously across the 8 cores (1250
each, padded to 1280).  Each core computes A-rows for its own nodes
(dst-sorted edge chunks; gather via dma_gather + segment-sum via PE matmul
with per-chunk sparse selection matrices), then an AllGather replicates
the level output for the next hop's gathers.  The dense GRU/readout phase
is node-local and runs entirely on-core.
"""

import sys
import types

sys.path.insert(0, "/opt/trn_rl_repo")

import numpy as np

N, F, P, E = 10000, 128, 4, 160000
C, HID = 512, 256
CORES = 8
M = N // CORES          # 1250 real nodes per core
MP = 1280               # padded nodes per core
NP = CORES * MP         # 10240 padded global nodes
TILES = MP // 128       # 10 dst tiles per core
GSZ = 6                 # segment-sum chunks (of 128 edges) per dma_gather
WCOLS = GSZ * 128 // 16  # int16 index columns per gather

TRACE = False           # set kernel.TRACE=True before calling for profiling
LAST_RESULT = {}        # exec_time_ns etc. for test harness introspection

_CACHE = {}


def _install_ntff_hook():
    """trace=True under axon needs antenv.axon_hooks, absent in this image."""
    import antenv

    if "antenv.axon_hooks" in sys.modules:
        return
    mod = types.ModuleType("antenv.axon_hooks")
    _h = [None]
    mod.set_axon_ntff_profile_hook = lambda h: _h.__setitem__(0, h)
    mod.get_axon_ntff_profile_hook = lambda: _h[0]
    sys.modules["antenv.axon_hooks"] = mod
    antenv.axon_hooks = mod
    try:
        from trn_agent_boot.trn_boot import _ntff_profile_via_ctypes

        mod.set_axon_ntff_profile_hook(
            _ntff_profile_via_ctypes("/opt/axon/libaxon_pjrt.so")
        )
    except Exception:
        pass


def _build_program(KCH, NG, need_bias, probs, b2v):
    import concourse.bacc as bacc
    import concourse.mybir as mybir
    from concourse import tile
    from concourse.masks import make_identity
    from contextlib import ExitStack

    f32 = mybir.dt.float32
    f32r = mybir.dt.float32r
    i16 = mybir.dt.int16
    AF = mybir.ActivationFunctionType
    OP = mybir.AluOpType

    nc = bacc.Bacc(
        "TRN2",
        target_bir_lowering=False,
        debug=False,
        enable_asserts=True,
        num_devices=CORES,
    )
    xpk = nc.dram_tensor("xpk", [NP, C], f32, kind="ExternalInput")
    sblk = nc.dram_tensor("sblk", [128, TILES * KCH * 128], f32, kind="ExternalInput")
    gidx = nc.dram_tensor("gidx", [128, TILES * NG * WCOLS], i16, kind="ExternalInput")
    wcat = nc.dram_tensor("wcat", [128, 4 * C], f32, kind="ExternalInput")
    lcat = nc.dram_tensor("lcat", [128, 12 * C], f32, kind="ExternalInput")
    lw1 = nc.dram_tensor("lw1", [128, 4 * HID], f32, kind="ExternalInput")
    lw2 = nc.dram_tensor("lw2", [128, 2], f32, kind="ExternalInput")
    biasin = nc.dram_tensor("biasin", [128, 16], f32, kind="ExternalInput")
    out_h = nc.dram_tensor("out_h", [MP, C], f32, kind="ExternalOutput")
    out_y = nc.dram_tensor("out_y", [MP, 1], f32, kind="ExternalOutput")

    with tile.TileContext(nc) as tc, ExitStack() as ctx:
        dram = ctx.enter_context(tc.tile_pool(name="dram", bufs=1, space="DRAM"))
        yl = [
            dram.tile([MP, C], f32, name=f"yl{L}", tag=f"yl{L}") for L in range(5)
        ]
        yf = [
            dram.tile([NP, C], f32, name=f"yf{L}", tag=f"yf{L}", addr_space="Shared")
            for L in range(4)
        ]
        p0 = ctx.enter_context(tc.tile_pool(name="p0", bufs=1))
        gidx_t = p0.tile([128, TILES * NG * WCOLS], i16)
        ident = p0.tile([128, 128], f32)
        nc.sync.dma_start(gidx_t[:], gidx[:, :])
        make_identity(nc, ident[:])

        nreg = nc.gpsimd.to_reg(GSZ * 128)

        # ---------------- message passing: Y_{k+1} = A Y_k ----------------
        with tc.tile_pool(name="msg", bufs=1) as pm, tc.tile_pool(
            name="mpsum", bufs=2, space="PSUM"
        ) as mpsum:
            s_t = pm.tile([128, TILES * KCH * 128], f32, bufs=1)
            nc.sync.dma_start(s_t[:], sblk[:, :])
            for L in range(5):
                src = xpk if L == 0 else yf[L - 1]
                for t in range(TILES):
                    yp = mpsum.tile([128, C], f32, tag="ypsum", name="yp")
                    for g in range(NG):
                        G = pm.tile(
                            [128, GSZ * C], f32, tag="G", bufs=4, name="G"
                        )
                        cb = (t * NG + g) * WCOLS
                        nc.gpsimd.dma_gather(
                            out_ap=G[:].rearrange("p (c e) -> p c e", e=C),
                            in_ap=src[:, :],
                            idxs_ap=gidx_t[:, cb : cb + WCOLS],
                            num_idxs=GSZ * 128,
                            num_idxs_reg=nreg,
                            elem_size=C,
                        )
                        for k in range(GSZ):
                            ch = g * GSZ + k
                            scol = (t * KCH + ch) * 128
                            nc.tensor.matmul(
                                yp[:],
                                lhsT=s_t[:, scol : scol + 128],
                                rhs=G[:, k * C : (k + 1) * C],
                                start=(ch == 0),
                                stop=(ch == KCH - 1),
                            )
                    ysb = pm.tile([128, C], f32, tag="ysb", bufs=3, name="ysb")
                    nc.vector.tensor_copy(ysb[:], yp[:])
                    nc.sync.dma_start(yl[L][t * 128 : (t + 1) * 128, :], ysb[:])
                if L < 4:
                    nc.gpsimd.collective_compute(
                        "AllGather",
                        mybir.AluOpType.bypass,
                        replica_groups=[list(range(CORES))],
                        ins=[yl[L][:, :].opt()],
                        outs=[yf[L][:, :].opt()],
                    )

        # ---------------- dense node-local phase ----------------
        with tc.tile_pool(name="dn", bufs=1) as pd, tc.tile_pool(
            name="dpsum", bufs=2, space="PSUM"
        ) as dpsum:
            wcat_t = pd.tile([128, 4 * C], f32r, bufs=1)
            lcat_t = pd.tile([128, 12 * C], f32r, bufs=1)
            lw1_t = pd.tile([128, 4 * HID], f32r, bufs=1)
            lw2_t = pd.tile([128, 2], f32r, bufs=1)
            bias_t = pd.tile([128, 16], f32, bufs=1)
            nc.gpsimd.dma_start(wcat_t[:], wcat[:, :])
            nc.gpsimd.dma_start(lcat_t[:], lcat[:, :])
            nc.gpsimd.dma_start(lw1_t[:], lw1[:, :])
            nc.gpsimd.dma_start(lw2_t[:], lw2[:, :])
            nc.sync.dma_start(bias_t[:], biasin[:, :])

            for ng0, nt in [(0, 512), (512, 512), (1024, 256)]:
                nb = nt // 128
                # transpose Y1/Y5 node-rows into feature-major tiles
                yT = {}
                for nm, ylsrc in [("y1", yl[0]), ("y5", yl[4])]:
                    for k4 in range(4):
                        yT[(nm, k4)] = pd.tile(
                            [128, nt], f32r, tag=f"{nm}T{k4}", bufs=1, name="yTt"
                        )
                    for b in range(nb):
                        rt = pd.tile([128, C], f32, tag="rt", bufs=3, name="rt")
                        nc.sync.dma_start(
                            rt[:], ylsrc[ng0 + b * 128 : ng0 + (b + 1) * 128, :]
                        )
                        for k4 in range(4):
                            tp = dpsum.tile(
                                [128, 128], f32, tag="tp", name="tp"
                            )
                            nc.tensor.transpose(
                                tp[:], rt[:, k4 * 128 : (k4 + 1) * 128], ident[:]
                            )
                            nc.vector.tensor_copy(
                                yT[(nm, k4)][:, b * 128 : (b + 1) * 128], tp[:]
                            )
                accs = [None] * 4
                for p in range(P):
                    H = [
                        pd.tile([128, nt], f32r, tag=f"H{m}", bufs=2, name="Ht_")
                        for m in range(4)
                    ]
                    for m in range(4):
                        hp = dpsum.tile([128, nt], f32, tag="dp", name="hp")
                        nc.tensor.matmul(
                            hp[:],
                            lhsT=wcat_t[:, m * 128 : (m + 1) * 128],
                            rhs=yT[("y5", p)][:],
                            start=True,
                            stop=True,
                        )
                        nc.vector.tensor_copy(H[m][:], hp[:])
                    gates = []
                    for gi, func in [(1, AF.Sigmoid), (2, AF.Sigmoid)]:
                        gt = [
                            pd.tile(
                                [128, nt], f32, tag=f"g{gi}{m}", bufs=1, name="gt"
                            )
                            for m in range(4)
                        ]
                        for m in range(4):
                            zp = dpsum.tile([128, nt], f32, tag="dp", name="zp")
                            nc.tensor.matmul(
                                zp[:],
                                lhsT=wcat_t[:, gi * C + m * 128 : gi * C + (m + 1) * 128],
                                rhs=yT[("y1", p)][:],
                                start=True,
                                stop=False,
                            )
                            Lsec = (gi - 1) * 4 * C
                            for k in range(4):
                                nc.tensor.matmul(
                                    zp[:],
                                    lhsT=lcat_t[
                                        :,
                                        Lsec + k * C + m * 128 : Lsec + k * C + (m + 1) * 128,
                                    ],
                                    rhs=H[k][:],
                                    start=False,
                                    stop=(k == 3),
                                )
                            bcol = (gi - 1) * 4 + m
                            if need_bias:
                                nc.scalar.activation(
                                    gt[m][:], zp[:], func,
                                    bias=bias_t[:, bcol : bcol + 1],
                                )
                            else:
                                nc.scalar.activation(gt[m][:], zp[:], func)
                        gates.append(gt)
                    Z, Rg = gates
                    HR = [
                        pd.tile([128, nt], f32r, tag=f"HR{m}", bufs=1, name="HRt")
                        for m in range(4)
                    ]
                    for m in range(4):
                        nc.vector.tensor_tensor(
                            out=HR[m][:], in0=H[m][:].bitcast(f32),
                            in1=Rg[m][:], op=OP.mult,
                        )
                    newacc = [None] * 4
                    for m in range(4):
                        tp2 = dpsum.tile([128, nt], f32, tag="dp", name="tp2")
                        nc.tensor.matmul(
                            tp2[:],
                            lhsT=wcat_t[:, 3 * C + m * 128 : 3 * C + (m + 1) * 128],
                            rhs=yT[("y1", p)][:],
                            start=True,
                            stop=False,
                        )
                        for k in range(4):
                            nc.tensor.matmul(
                                tp2[:],
                                lhsT=lcat_t[
                                    :,
                                    8 * C + k * C + m * 128 : 8 * C + k * C + (m + 1) * 128,
                                ],
                                rhs=HR[k][:],
                                start=False,
                                stop=(k == 3),
                            )
                        Htn = pd.tile([128, nt], f32, tag="Htn", bufs=2, name="Htn")
                        if need_bias:
                            nc.scalar.activation(
                                Htn[:], tp2[:], AF.Tanh, bias=bias_t[:, 8 + m : 9 + m]
                            )
                        else:
                            nc.scalar.activation(Htn[:], tp2[:], AF.Tanh)
                        d1 = pd.tile([128, nt], f32, tag="d1", bufs=2, name="d1")
                        nc.vector.tensor_tensor(
                            out=d1[:], in0=H[m][:].bitcast(f32), in1=Htn[:],
                            op=OP.subtract,
                        )
                        d2 = pd.tile([128, nt], f32, tag="d2", bufs=2, name="d2")
                        nc.vector.tensor_tensor(
                            out=d2[:], in0=Z[m][:], in1=d1[:], op=OP.mult
                        )
                        hn = pd.tile([128, nt], f32, tag="hn", bufs=2, name="hn")
                        nc.vector.tensor_tensor(
                            out=hn[:], in0=d2[:], in1=Htn[:], op=OP.add
                        )
                        na = pd.tile([128, nt], f32, tag=f"acc{m}", bufs=2, name="na")
                        if p == 0:
                            nc.vector.tensor_scalar_mul(na[:], hn[:], float(probs[0]))
                        else:
                            nc.vector.scalar_tensor_tensor(
                                out=na[:], in0=hn[:], scalar=float(probs[p]),
                                in1=accs[m][:], op0=OP.mult, op1=OP.add,
                            )
                        newacc[m] = na
                    accs = newacc
                # readout
                rl = [
                    pd.tile([128, nt], f32r, tag=f"rl{m}", bufs=1, name="rl")
                    for m in range(4)
                ]
                for m in range(4):
                    nc.scalar.activation(rl[m][:], accs[m][:], AF.Relu)
                h1 = []
                for hm in range(2):
                    pp = dpsum.tile([128, nt], f32, tag="dp", name="pp")
                    for k in range(4):
                        nc.tensor.matmul(
                            pp[:],
                            lhsT=lw1_t[:, k * HID + hm * 128 : k * HID + (hm + 1) * 128],
                            rhs=rl[k][:],
                            start=(k == 0),
                            stop=(k == 3),
                        )
                    t1 = pd.tile([128, nt], f32r, tag=f"h1{hm}", bufs=1, name="t1")
                    if need_bias:
                        nc.scalar.activation(
                            t1[:], pp[:], AF.Relu, bias=bias_t[:, 12 + hm : 13 + hm]
                        )
                    else:
                        nc.scalar.activation(t1[:], pp[:], AF.Relu)
                    h1.append(t1)
                py = dpsum.tile([1, nt], f32, tag="py", name="py")
                for hm in range(2):
                    nc.tensor.matmul(
                        py[:],
                        lhsT=lw2_t[:, hm : hm + 1],
                        rhs=h1[hm][:],
                        start=(hm == 0),
                        stop=(hm == 1),
                    )
                oy = pd.tile([1, nt], f32, tag="oy", bufs=2, name="oy")
                nc.vector.tensor_scalar_add(oy[:], py[:], float(b2v))
                nc.sync.dma_start(
                    out_y[ng0 : ng0 + nt, 0:1].rearrange("a b -> b a"),
                    oy[0:1, :nt],
                )
                for m in range(4):
                    for b in range(nb):
                        tp = dpsum.tile([128, 128], f32, tag="tp", name="tp3")
                        nc.tensor.transpose(
                            tp[:], accs[m][:, b * 128 : (b + 1) * 128], ident[:]
                        )
                        hsb = pd.tile([128, 128], f32, tag="hsb", bufs=3, name="hsb")
                        nc.vector.tensor_copy(hsb[:], tp[:])
                        nc.sync.dma_start(
                            out_h[
                                ng0 + b * 128 : ng0 + (b + 1) * 128,
                                m * 128 : (m + 1) * 128,
                            ],
                            hsb[:],
                        )
    nc.compile()
    return nc


def kernel(**inputs):
    x = np.asarray(inputs["x"], np.float32)
    edge_index = np.asarray(inputs["edge_index"])
    edge_attr = np.asarray(inputs["edge_attr"], np.float32)

    # ---- graph preprocessing (host): norm, partition, dst-sorted chunks ----
    src = np.concatenate([edge_index[0], np.arange(N)]).astype(np.int64)
    dst = np.concatenate([edge_index[1], np.arange(N)]).astype(np.int64)
    ew = np.concatenate([edge_attr, np.ones(N, np.float32)]).astype(np.float32)
    deg = np.zeros(N, np.float32)
    np.add.at(deg, dst, ew)
    dinv = np.where(deg > 0, 1.0 / np.sqrt(np.where(deg > 0, deg, 1.0)), 0.0).astype(
        np.float32
    )
    norm = (dinv[src] * ew * dinv[dst]).astype(np.float32)
    core_of = dst // M
    dst_local = dst % M
    src_pad = ((src // M) * MP + (src % M)).astype(np.int64)

    # per (core, tile) edge lists
    per_ct = [[None] * TILES for _ in range(CORES)]
    counts = np.zeros((CORES, TILES), np.int64)
    tile_of = dst_local // 128
    key = core_of * TILES + tile_of
    order = np.argsort(key * (M + 1) + dst_local, kind="stable")
    s_src, s_norm, s_dl, s_key = (
        src_pad[order],
        norm[order],
        dst_local[order],
        key[order],
    )
    bounds = np.searchsorted(s_key, np.arange(CORES * TILES + 1))
    for c in range(CORES):
        for t in range(TILES):
            a, b = bounds[c * TILES + t], bounds[c * TILES + t + 1]
            per_ct[c][t] = (s_src[a:b], s_norm[a:b], s_dl[a:b] - t * 128)
            counts[c, t] = b - a
    KCH_need = int(np.max((counts + 127) // 128))
    NG = (KCH_need + GSZ - 1) // GSZ
    KCH = NG * GSZ

    sblks = []
    gidxs = []
    for c in range(CORES):
        S = np.zeros((128, TILES * KCH, 128), np.float32)
        IDX = np.zeros((TILES * KCH * 128,), np.int16)
        for t in range(TILES):
            e_src, e_norm, e_d = per_ct[c][t]
            n = len(e_src)
            base = t * KCH * 128
            sl = np.arange(n)
            S[sl % 128, t * KCH + sl // 128, e_d] = e_norm
            IDX[base : base + n] = e_src.astype(np.int16)
        # wrap indices per gather: [16, WCOLS] blocks replicated to 128 parts
        gi = np.zeros((128, TILES * NG * WCOLS), np.int16)
        for t in range(TILES):
            for g in range(NG):
                a = t * KCH * 128 + g * GSZ * 128
                arr = IDX[a : a + GSZ * 128]
                blk = arr.reshape(WCOLS, 16).T  # [16, WCOLS]
                gi[:, (t * NG + g) * WCOLS : (t * NG + g + 1) * WCOLS] = np.tile(
                    blk, (8, 1)
                )
        sblks.append(S.reshape(128, TILES * KCH * 128))
        gidxs.append(gi)

    # ---- weight composition (host, fp64 -> fp32) ----
    W = [np.asarray(inputs[f"W{i}"], np.float64) for i in range(1, 6)]
    bvec = [np.asarray(inputs[f"b{i}"], np.float64) for i in range(1, 6)]
    Lz = np.asarray(inputs["Lz_w"], np.float64)
    Lr = np.asarray(inputs["Lr_w"], np.float64)
    Lh = np.asarray(inputs["Lh_w"], np.float64)
    Wc = W[0]
    for Wi in W[1:]:
        Wc = Wc @ Wi
    Wzp = np.asarray(inputs["Wz"], np.float64) @ Lz[:C]
    Wrp = np.asarray(inputs["Wr"], np.float64) @ Lr[:C]
    Whp = np.asarray(inputs["Wh"], np.float64) @ Lh[:C]
    wcat = np.concatenate([Wc, Wzp, Wrp, Whp], axis=1).astype(np.float32)

    def chunkrows(A):  # [512, X] -> [128, 4*X]
        return np.concatenate([A[k * 128 : (k + 1) * 128] for k in range(4)], 1)

    lcat = np.concatenate(
        [
            chunkrows(Lz[C:].astype(np.float32)),
            chunkrows(Lr[C:].astype(np.float32)),
            chunkrows(Lh[C:].astype(np.float32)),
        ],
        axis=1,
    )
    lw1 = chunkrows(np.asarray(inputs["lin1_w"], np.float32))
    lw2m = np.asarray(inputs["lin2_w"], np.float32)  # [256, 1]
    lw2 = np.stack([lw2m[:128, 0], lw2m[128:, 0]], axis=1)  # [128, 2]

    bz = np.asarray(inputs["bz"], np.float64) @ Lz[:C] + np.asarray(
        inputs["Lz_b"], np.float64
    )
    br = np.asarray(inputs["br"], np.float64) @ Lr[:C] + np.asarray(
        inputs["Lr_b"], np.float64
    )
    bh = np.asarray(inputs["bh"], np.float64) @ Lh[:C] + np.asarray(
        inputs["Lh_b"], np.float64
    )
    lin1_b = np.asarray(inputs["lin1_b"], np.float64)
    biasarr = np.zeros((128, 16), np.float32)
    for mm in range(4):
        biasarr[:, mm] = bz[mm * 128 : (mm + 1) * 128]
        biasarr[:, 4 + mm] = br[mm * 128 : (mm + 1) * 128]
        biasarr[:, 8 + mm] = bh[mm * 128 : (mm + 1) * 128]
    biasarr[:, 12] = lin1_b[:128]
    biasarr[:, 13] = lin1_b[128:]
    need_bias = bool(np.abs(biasarr).max() > 0)
    # stacked-conv bias correction must be zero for the composed-weight path
    assert all(np.abs(b).max() == 0 for b in bvec), "nonzero conv bias unsupported"

    attn = np.asarray(inputs["attn"], np.float64)
    probs = np.exp(attn - attn.max())
    probs = (probs / probs.sum()).astype(np.float32)
    b2v = float(np.asarray(inputs["lin2_b"])[0])

    # ---- packed node features, padded layout ----
    xpk = np.zeros((NP, C), np.float32)
    xr = x.transpose(0, 2, 1).reshape(N, P * F)
    for c in range(CORES):
        xpk[c * MP : c * MP + M] = xr[c * M : (c + 1) * M]

    # ---- build / fetch program ----
    key2 = (KCH, NG, need_bias, tuple(np.round(probs, 7)), round(b2v, 7))
    if key2 not in _CACHE:
        _CACHE[key2] = _build_program(KCH, NG, need_bias, probs, b2v)
    nc = _CACHE[key2]

    from concourse.bass_utils import run_bass_kernel_spmd

    in_maps = []
    for c in range(CORES):
        in_maps.append(
            {
                "xpk": xpk,
                "sblk": sblks[c],
                "gidx": gidxs[c],
                "wcat": wcat,
                "lcat": lcat,
                "lw1": lw1,
                "lw2": lw2,
                "biasin": biasarr,
            }
        )
    if TRACE:
        _install_ntff_hook()
    res = run_bass_kernel_spmd(
        nc, in_maps, core_ids=list(range(CORES)), trace=TRACE
    )
    LAST_RESULT["exec_time_ns"] = res.exec_time_ns
    LAST_RESULT["res"] = res

    out_hidden = np.concatenate(
        [res.results[c]["out_h"][:M] for c in range(CORES)], axis=0
    )
    out = np.concatenate([res.results[c]["out_y"][:M] for c in range(CORES)], axis=0)
    return out, out_hidden


# revision 18
# speedup vs baseline: 1.1667x; 1.0863x over previous
"""Trainium2 Bass kernel for nn_ConvStackedTemporalGCN (8 NeuronCores, SPMD).

Strategy
--------
The reference network is, per period p:
    5 stacked GCNConv (linear, no activation between) -> H
    3 gate GCNConvs on Xp feeding a GRU-style cell    -> Hn
    H_accum += softmax(attn)[p] * Hn
then a node-local MLP readout.

Because gcn(h,W,b) = A(hW) + b with a FIXED normalized adjacency A, and A
(row mixing) commutes with W (column mixing), the 5 stacked convs collapse
to  H = (A^5 Xp) (W1..W5) + bias-terms, and the gate convs share Y1 = A Xp.
All 4 periods are packed into the feature dim (X [N, 4*128]), so the whole
message-passing phase is just FIVE sparse A-applications on [N, 512] data.

Sharding: nodes are partitioned contiguously across the 8 cores (1250
each, padded to 1280).  Each core computes A-rows for its own nodes
(dst-sorted edge chunks; gather via dma_gather + segment-sum via PE matmul
with per-chunk sparse selection matrices), then an AllGather replicates
the level output for the next hop's gathers.  The dense GRU/readout phase
is node-local and runs entirely on-core.
"""

import sys
import types

sys.path.insert(0, "/opt/trn_rl_repo")

import numpy as np

N, F, P, E = 10000, 128, 4, 160000
C, HID = 512, 256
CORES = 8
M = N // CORES          # 1250 real nodes per core
MP = 1280               # padded nodes per core
NP = CORES * MP         # 10240 padded global nodes
TILES = MP // 128       # 10 dst tiles per core
GSZ = 6                 # segment-sum chunks (of 128 edges) per dma_gather
WCOLS = GSZ * 128 // 16  # int16 index columns per gather

TRACE = False           # set kernel.TRACE=True before calling for profiling
LAST_RESULT = {}        # exec_time_ns etc. for test harness introspection

_CACHE = {}


def _install_ntff_hook():
    """trace=True under axon needs antenv.axon_hooks, absent in this image."""
    import antenv

    if "antenv.axon_hooks" in sys.modules:
        return
    mod = types.ModuleType("antenv.axon_hooks")
    _h = [None]
    mod.set_axon_ntff_profile_hook = lambda h: _h.__setitem__(0, h)
    mod.get_axon_ntff_profile_hook = lambda: _h[0]
    sys.modules["antenv.axon_hooks"] = mod
    antenv.axon_hooks = mod
    try:
        from trn_agent_boot.trn_boot import _ntff_profile_via_ctypes

        mod.set_axon_ntff_profile_hook(
            _ntff_profile_via_ctypes("/opt/axon/libaxon_pjrt.so")
        )
    except Exception:
        pass


def _build_program(KCH, NG, need_bias, probs, b2v):
    import concourse.bacc as bacc
    import concourse.mybir as mybir
    from concourse import tile
    from concourse.masks import make_identity
    from contextlib import ExitStack

    f32 = mybir.dt.float32
    f32r = mybir.dt.float32r
    i16 = mybir.dt.int16
    AF = mybir.ActivationFunctionType
    OP = mybir.AluOpType

    nc = bacc.Bacc(
        "TRN2",
        target_bir_lowering=False,
        debug=False,
        enable_asserts=True,
        num_devices=CORES,
    )
    xpk = nc.dram_tensor("xpk", [NP, C], f32, kind="ExternalInput")
    sblk = nc.dram_tensor("sblk", [128, TILES * KCH * 128], f32, kind="ExternalInput")
    gidx = nc.dram_tensor("gidx", [128, TILES * NG * WCOLS], i16, kind="ExternalInput")
    wcat = nc.dram_tensor("wcat", [128, 4 * C], f32, kind="ExternalInput")
    lcat = nc.dram_tensor("lcat", [128, 12 * C], f32, kind="ExternalInput")
    lw1 = nc.dram_tensor("lw1", [128, 4 * HID], f32, kind="ExternalInput")
    lw2 = nc.dram_tensor("lw2", [128, 2], f32, kind="ExternalInput")
    biasin = nc.dram_tensor("biasin", [128, 16], f32, kind="ExternalInput")
    out_h = nc.dram_tensor("out_h", [MP, C], f32, kind="ExternalOutput")
    out_y = nc.dram_tensor("out_y", [MP, 1], f32, kind="ExternalOutput")

    with tile.TileContext(nc) as tc, ExitStack() as ctx:
        dram = ctx.enter_context(tc.tile_pool(name="dram", bufs=1, space="DRAM"))
        yl = [
            dram.tile([MP, C], f32, name=f"yl{L}", tag=f"yl{L}") for L in range(5)
        ]
        yf = [
            dram.tile([NP, C], f32, name=f"yf{L}", tag=f"yf{L}", addr_space="Shared")
            for L in range(4)
        ]
        xr = dram.tile([NP, C], f32, name="xr", tag="xr")
        p0 = ctx.enter_context(tc.tile_pool(name="p0", bufs=1))
        gidx_t = p0.tile([128, TILES * NG * WCOLS], i16)
        ident = p0.tile([128, 128], f32)
        identr = p0.tile([128, 128], f32r)
        nc.sync.dma_start(gidx_t[:], gidx[:, :])
        make_identity(nc, ident[:])
        nc.gpsimd.dma_start(identr[:], ident[:])
        nc.gpsimd.dma_start(xr[:, :].bitcast(f32r), xpk[:, :])  # round once

        nreg = nc.gpsimd.to_reg(GSZ * 128)

        # ---------------- message passing: Y_{k+1} = A Y_k ----------------
        with tc.tile_pool(name="msg", bufs=1) as pm, tc.tile_pool(
            name="mpsum", bufs=2, space="PSUM"
        ) as mpsum:
            s_t = pm.tile([128, TILES * KCH * 128], f32r, bufs=1)
            nc.gpsimd.dma_start(s_t[:], sblk[:, :])
            for L in range(5):
                src = xr if L == 0 else yf[L - 1]
                for t in range(TILES):
                    yp = mpsum.tile([128, C], f32, tag="ypsum", name="yp")
                    for g in range(NG):
                        G = pm.tile(
                            [128, GSZ * C], f32r, tag="G", bufs=3, name="G"
                        )
                        cb = (t * NG + g) * WCOLS
                        nc.gpsimd.dma_gather(
                            out_ap=G[:].rearrange("p (c e) -> p c e", e=C),
                            in_ap=src[:, :].bitcast(f32r),
                            idxs_ap=gidx_t[:, cb : cb + WCOLS],
                            num_idxs=GSZ * 128,
                            num_idxs_reg=nreg,
                            elem_size=C,
                        )
                        for k in range(GSZ):
                            ch = g * GSZ + k
                            scol = (t * KCH + ch) * 128
                            nc.tensor.matmul(
                                yp[:],
                                lhsT=s_t[:, scol : scol + 128],
                                rhs=G[:, k * C : (k + 1) * C],
                                start=(ch == 0),
                                stop=(ch == KCH - 1),
                            )
                    ysb = pm.tile([128, C], f32r, tag="ysb", bufs=3, name="ysb")
                    nc.vector.tensor_copy(ysb[:], yp[:])
                    nc.sync.dma_start(yl[L][t * 128 : (t + 1) * 128, :], ysb[:].bitcast(f32))
                if L < 4:
                    nc.gpsimd.collective_compute(
                        "AllGather",
                        mybir.AluOpType.bypass,
                        replica_groups=[list(range(CORES))],
                        ins=[yl[L][:, :].opt()],
                        outs=[yf[L][:, :].opt()],
                    )

        # ---------------- dense node-local phase ----------------
        with tc.tile_pool(name="dn", bufs=1) as pd, tc.tile_pool(
            name="dpsum", bufs=2, space="PSUM"
        ) as dpsum:
            wcat_t = pd.tile([128, 4 * C], f32r, bufs=1)
            lcat_t = pd.tile([128, 12 * C], f32r, bufs=1)
            lw1_t = pd.tile([128, 4 * HID], f32r, bufs=1)
            lw2_t = pd.tile([128, 2], f32r, bufs=1)
            bias_t = pd.tile([128, 16], f32, bufs=1)
            nc.gpsimd.dma_start(wcat_t[:], wcat[:, :])
            nc.gpsimd.dma_start(lcat_t[:], lcat[:, :])
            nc.gpsimd.dma_start(lw1_t[:], lw1[:, :])
            nc.gpsimd.dma_start(lw2_t[:], lw2[:, :])
            nc.sync.dma_start(bias_t[:], biasin[:, :])

            for ng0, nt in [(0, 512), (512, 512), (1024, 256)]:
                nb = nt // 128
                # transpose Y1/Y5 node-rows into feature-major tiles
                yT = {}
                for nm, ylsrc in [("y1", yl[0]), ("y5", yl[4])]:
                    for k4 in range(4):
                        yT[(nm, k4)] = pd.tile(
                            [128, nt], f32r, tag=f"{nm}T{k4}", bufs=1, name="yTt"
                        )
                    for b in range(nb):
                        rt = pd.tile([128, C], f32r, tag="rt", bufs=3, name="rt")
                        nc.sync.dma_start(
                            rt[:],
                            ylsrc[ng0 + b * 128 : ng0 + (b + 1) * 128, :].bitcast(f32r),
                        )
                        for k4 in range(4):
                            tp = dpsum.tile(
                                [128, 128], f32r, tag="tp", name="tp"
                            )
                            nc.tensor.transpose(
                                tp[:], rt[:, k4 * 128 : (k4 + 1) * 128], identr[:]
                            )
                            nc.vector.tensor_copy(
                                yT[(nm, k4)][:, b * 128 : (b + 1) * 128], tp[:]
                            )
                accs = [None] * 4
                for p in range(P):
                    H = [
                        pd.tile([128, nt], f32r, tag=f"H{m}", bufs=2, name="Ht_")
                        for m in range(4)
                    ]
                    for m in range(4):
                        hp = dpsum.tile([128, nt], f32, tag="dp", name="hp")
                        nc.tensor.matmul(
                            hp[:],
                            lhsT=wcat_t[:, m * 128 : (m + 1) * 128],
                            rhs=yT[("y5", p)][:],
                            start=True,
                            stop=True,
                        )
                        nc.vector.tensor_copy(H[m][:], hp[:])
                    gates = []
                    for gi, func in [(1, AF.Sigmoid), (2, AF.Sigmoid)]:
                        gt = [
                            pd.tile(
                                [128, nt], f32, tag=f"g{gi}{m}", bufs=1, name="gt"
                            )
                            for m in range(4)
                        ]
                        for m in range(4):
                            zp = dpsum.tile([128, nt], f32, tag="dp", name="zp")
                            nc.tensor.matmul(
                                zp[:],
                                lhsT=wcat_t[:, gi * C + m * 128 : gi * C + (m + 1) * 128],
                                rhs=yT[("y1", p)][:],
                                start=True,
                                stop=False,
                            )
                            Lsec = (gi - 1) * 4 * C
                            for k in range(4):
                                nc.tensor.matmul(
                                    zp[:],
                                    lhsT=lcat_t[
                                        :,
                                        Lsec + k * C + m * 128 : Lsec + k * C + (m + 1) * 128,
                                    ],
                                    rhs=H[k][:],
                                    start=False,
                                    stop=(k == 3),
                                )
                            bcol = (gi - 1) * 4 + m
                            if need_bias:
                                nc.scalar.activation(
                                    gt[m][:], zp[:], func,
                                    bias=bias_t[:, bcol : bcol + 1],
                                )
                            else:
                                nc.scalar.activation(gt[m][:], zp[:], func)
                        gates.append(gt)
                    Z, Rg = gates
                    HR = [
                        pd.tile([128, nt], f32r, tag=f"HR{m}", bufs=1, name="HRt")
                        for m in range(4)
                    ]
                    for m in range(4):
                        nc.vector.tensor_tensor(
                            out=HR[m][:], in0=H[m][:].bitcast(f32),
                            in1=Rg[m][:], op=OP.mult,
                        )
                    newacc = [None] * 4
                    for m in range(4):
                        tp2 = dpsum.tile([128, nt], f32, tag="dp", name="tp2")
                        nc.tensor.matmul(
                            tp2[:],
                            lhsT=wcat_t[:, 3 * C + m * 128 : 3 * C + (m + 1) * 128],
                            rhs=yT[("y1", p)][:],
                            start=True,
                            stop=False,
                        )
                        for k in range(4):
                            nc.tensor.matmul(
                                tp2[:],
                                lhsT=lcat_t[
                                    :,
                                    8 * C + k * C + m * 128 : 8 * C + k * C + (m + 1) * 128,
                                ],
                                rhs=HR[k][:],
                                start=False,
                                stop=(k == 3),
                            )
                        Htn = pd.tile([128, nt], f32, tag="Htn", bufs=2, name="Htn")
                        if need_bias:
                            nc.scalar.activation(
                                Htn[:], tp2[:], AF.Tanh, bias=bias_t[:, 8 + m : 9 + m]
                            )
                        else:
                            nc.scalar.activation(Htn[:], tp2[:], AF.Tanh)
                        d1 = pd.tile([128, nt], f32, tag="d1", bufs=2, name="d1")
                        nc.vector.tensor_tensor(
                            out=d1[:], in0=H[m][:].bitcast(f32), in1=Htn[:],
                            op=OP.subtract,
                        )
                        d2 = pd.tile([128, nt], f32, tag="d2", bufs=2, name="d2")
                        nc.vector.tensor_tensor(
                            out=d2[:], in0=Z[m][:], in1=d1[:], op=OP.mult
                        )
                        hn = pd.tile([128, nt], f32, tag="hn", bufs=2, name="hn")
                        nc.vector.tensor_tensor(
                            out=hn[:], in0=d2[:], in1=Htn[:], op=OP.add
                        )
                        na = pd.tile([128, nt], f32, tag=f"acc{m}", bufs=2, name="na")
                        if p == 0:
                            nc.vector.tensor_scalar_mul(na[:], hn[:], float(probs[0]))
                        else:
                            nc.vector.scalar_tensor_tensor(
                                out=na[:], in0=hn[:], scalar=float(probs[p]),
                                in1=accs[m][:], op0=OP.mult, op1=OP.add,
                            )
                        newacc[m] = na
                    accs = newacc
                # readout
                rl = [
                    pd.tile([128, nt], f32r, tag=f"rl{m}", bufs=1, name="rl")
                    for m in range(4)
                ]
                for m in range(4):
                    nc.scalar.activation(rl[m][:], accs[m][:], AF.Relu)
                h1 = []
                for hm in range(2):
                    pp = dpsum.tile([128, nt], f32, tag="dp", name="pp")
                    for k in range(4):
                        nc.tensor.matmul(
                            pp[:],
                            lhsT=lw1_t[:, k * HID + hm * 128 : k * HID + (hm + 1) * 128],
                            rhs=rl[k][:],
                            start=(k == 0),
                            stop=(k == 3),
                        )
                    t1 = pd.tile([128, nt], f32r, tag=f"h1{hm}", bufs=1, name="t1")
                    if need_bias:
                        nc.scalar.activation(
                            t1[:], pp[:], AF.Relu, bias=bias_t[:, 12 + hm : 13 + hm]
                        )
                    else:
                        nc.scalar.activation(t1[:], pp[:], AF.Relu)
                    h1.append(t1)
                py = dpsum.tile([1, nt], f32, tag="py", name="py")
                for hm in range(2):
                    nc.tensor.matmul(
                        py[:],
                        lhsT=lw2_t[:, hm : hm + 1],
                        rhs=h1[hm][:],
                        start=(hm == 0),
                        stop=(hm == 1),
                    )
                oy = pd.tile([1, nt], f32, tag="oy", bufs=2, name="oy")
                nc.vector.tensor_scalar_add(oy[:], py[:], float(b2v))
                nc.sync.dma_start(
                    out_y[ng0 : ng0 + nt, 0:1].rearrange("a b -> b a"),
                    oy[0:1, :nt],
                )
                for m in range(4):
                    for b in range(nb):
                        tp = dpsum.tile([128, 128], f32, tag="tp", name="tp3")
                        nc.tensor.transpose(
                            tp[:], accs[m][:, b * 128 : (b + 1) * 128], ident[:]
                        )
                        hsb = pd.tile([128, 128], f32, tag="hsb", bufs=3, name="hsb")
                        nc.vector.tensor_copy(hsb[:], tp[:])
                        nc.sync.dma_start(
                            out_h[
                                ng0 + b * 128 : ng0 + (b + 1) * 128,
                                m * 128 : (m + 1) * 128,
                            ],
                            hsb[:],
                        )
    nc.compile()
    return nc


def kernel(**inputs):
    x = np.asarray(inputs["x"], np.float32)
    edge_index = np.asarray(inputs["edge_index"])
    edge_attr = np.asarray(inputs["edge_attr"], np.float32)

    # ---- graph preprocessing (host): norm, partition, dst-sorted chunks ----
    src = np.concatenate([edge_index[0], np.arange(N)]).astype(np.int64)
    dst = np.concatenate([edge_index[1], np.arange(N)]).astype(np.int64)
    ew = np.concatenate([edge_attr, np.ones(N, np.float32)]).astype(np.float32)
    deg = np.zeros(N, np.float32)
    np.add.at(deg, dst, ew)
    dinv = np.where(deg > 0, 1.0 / np.sqrt(np.where(deg > 0, deg, 1.0)), 0.0).astype(
        np.float32
    )
    norm = (dinv[src] * ew * dinv[dst]).astype(np.float32)
    core_of = dst // M
    dst_local = dst % M
    src_pad = ((src // M) * MP + (src % M)).astype(np.int64)

    # per (core, tile) edge lists
    per_ct = [[None] * TILES for _ in range(CORES)]
    counts = np.zeros((CORES, TILES), np.int64)
    tile_of = dst_local // 128
    key = core_of * TILES + tile_of
    order = np.argsort(key * (M + 1) + dst_local, kind="stable")
    s_src, s_norm, s_dl, s_key = (
        src_pad[order],
        norm[order],
        dst_local[order],
        key[order],
    )
    bounds = np.searchsorted(s_key, np.arange(CORES * TILES + 1))
    for c in range(CORES):
        for t in range(TILES):
            a, b = bounds[c * TILES + t], bounds[c * TILES + t + 1]
            per_ct[c][t] = (s_src[a:b], s_norm[a:b], s_dl[a:b] - t * 128)
            counts[c, t] = b - a
    KCH_need = int(np.max((counts + 127) // 128))
    NG = (KCH_need + GSZ - 1) // GSZ
    KCH = NG * GSZ

    sblks = []
    gidxs = []
    for c in range(CORES):
        S = np.zeros((128, TILES * KCH, 128), np.float32)
        IDX = np.zeros((TILES * KCH * 128,), np.int16)
        for t in range(TILES):
            e_src, e_norm, e_d = per_ct[c][t]
            n = len(e_src)
            base = t * KCH * 128
            sl = np.arange(n)
            S[sl % 128, t * KCH + sl // 128, e_d] = e_norm
            IDX[base : base + n] = e_src.astype(np.int16)
        # wrap indices per gather: [16, WCOLS] blocks replicated to 128 parts
        gi = np.zeros((128, TILES * NG * WCOLS), np.int16)
        for t in range(TILES):
            for g in range(NG):
                a = t * KCH * 128 + g * GSZ * 128
                arr = IDX[a : a + GSZ * 128]
                blk = arr.reshape(WCOLS, 16).T  # [16, WCOLS]
                gi[:, (t * NG + g) * WCOLS : (t * NG + g + 1) * WCOLS] = np.tile(
                    blk, (8, 1)
                )
        sblks.append(S.reshape(128, TILES * KCH * 128))
        gidxs.append(gi)

    # ---- weight composition (host, fp64 -> fp32) ----
    W = [np.asarray(inputs[f"W{i}"], np.float64) for i in range(1, 6)]
    bvec = [np.asarray(inputs[f"b{i}"], np.float64) for i in range(1, 6)]
    Lz = np.asarray(inputs["Lz_w"], np.float64)
    Lr = np.asarray(inputs["Lr_w"], np.float64)
    Lh = np.asarray(inputs["Lh_w"], np.float64)
    Wc = W[0]
    for Wi in W[1:]:
        Wc = Wc @ Wi
    Wzp = np.asarray(inputs["Wz"], np.float64) @ Lz[:C]
    Wrp = np.asarray(inputs["Wr"], np.float64) @ Lr[:C]
    Whp = np.asarray(inputs["Wh"], np.float64) @ Lh[:C]
    wcat = np.concatenate([Wc, Wzp, Wrp, Whp], axis=1).astype(np.float32)

    def chunkrows(A):  # [512, X] -> [128, 4*X]
        return np.concatenate([A[k * 128 : (k + 1) * 128] for k in range(4)], 1)

    lcat = np.concatenate(
        [
            chunkrows(Lz[C:].astype(np.float32)),
            chunkrows(Lr[C:].astype(np.float32)),
            chunkrows(Lh[C:].astype(np.float32)),
        ],
        axis=1,
    )
    lw1 = chunkrows(np.asarray(inputs["lin1_w"], np.float32))
    lw2m = np.asarray(inputs["lin2_w"], np.float32)  # [256, 1]
    lw2 = np.stack([lw2m[:128, 0], lw2m[128:, 0]], axis=1)  # [128, 2]

    bz = np.asarray(inputs["bz"], np.float64) @ Lz[:C] + np.asarray(
        inputs["Lz_b"], np.float64
    )
    br = np.asarray(inputs["br"], np.float64) @ Lr[:C] + np.asarray(
        inputs["Lr_b"], np.float64
    )
    bh = np.asarray(inputs["bh"], np.float64) @ Lh[:C] + np.asarray(
        inputs["Lh_b"], np.float64
    )
    lin1_b = np.asarray(inputs["lin1_b"], np.float64)
    biasarr = np.zeros((128, 16), np.float32)
    for mm in range(4):
        biasarr[:, mm] = bz[mm * 128 : (mm + 1) * 128]
        biasarr[:, 4 + mm] = br[mm * 128 : (mm + 1) * 128]
        biasarr[:, 8 + mm] = bh[mm * 128 : (mm + 1) * 128]
    biasarr[:, 12] = lin1_b[:128]
    biasarr[:, 13] = lin1_b[128:]
    need_bias = bool(np.abs(biasarr).max() > 0)
    # stacked-conv bias correction must be zero for the composed-weight path
    assert all(np.abs(b).max() == 0 for b in bvec), "nonzero conv bias unsupported"

    attn = np.asarray(inputs["attn"], np.float64)
    probs = np.exp(attn - attn.max())
    probs = (probs / probs.sum()).astype(np.float32)
    b2v = float(np.asarray(inputs["lin2_b"])[0])

    # ---- packed node features, padded layout ----
    xpk = np.zeros((NP, C), np.float32)
    xr = x.transpose(0, 2, 1).reshape(N, P * F)
    for c in range(CORES):
        xpk[c * MP : c * MP + M] = xr[c * M : (c + 1) * M]

    # ---- build / fetch program ----
    key2 = (KCH, NG, need_bias, tuple(np.round(probs, 7)), round(b2v, 7))
    if key2 not in _CACHE:
        _CACHE[key2] = _build_program(KCH, NG, need_bias, probs, b2v)
    nc = _CACHE[key2]

    from concourse.bass_utils import run_bass_kernel_spmd

    in_maps = []
    for c in range(CORES):
        in_maps.append(
            {
                "xpk": xpk,
                "sblk": sblks[c],
                "gidx": gidxs[c],
                "wcat": wcat,
                "lcat": lcat,
                "lw1": lw1,
                "lw2": lw2,
                "biasin": biasarr,
            }
        )
    if TRACE:
        _install_ntff_hook()
    res = run_bass_kernel_spmd(
        nc, in_maps, core_ids=list(range(CORES)), trace=TRACE
    )
    LAST_RESULT["exec_time_ns"] = res.exec_time_ns
    LAST_RESULT["res"] = res

    out_hidden = np.concatenate(
        [res.results[c]["out_h"][:M] for c in range(CORES)], axis=0
    )
    out = np.concatenate([res.results[c]["out_y"][:M] for c in range(CORES)], axis=0)
    return out, out_hidden


# revision 20
# speedup vs baseline: 1.2378x; 1.0610x over previous
"""Trainium2 Bass kernel for nn_ConvStackedTemporalGCN (8 NeuronCores, SPMD).

Strategy
--------
The reference network is, per period p:
    5 stacked GCNConv (linear, no activation between) -> H
    3 gate GCNConvs on Xp feeding a GRU-style cell    -> Hn
    H_accum += softmax(attn)[p] * Hn
then a node-local MLP readout.

Because gcn(h,W,b) = A(hW) + b with a FIXED normalized adjacency A, and A
(row mixing) commutes with W (column mixing), the 5 stacked convs collapse
to  H = (A^5 Xp) (W1..W5) + bias-terms, and the gate convs share Y1 = A Xp.
All 4 periods are packed into the feature dim (X [N, 4*128]), so the whole
message-passing phase is just FIVE sparse A-applications on [N, 512] data.

Sharding: nodes are partitioned contiguously across the 8 cores (1250
each, padded to 1280).  Each core computes A-rows for its own nodes
(dst-sorted edge chunks; gather via dma_gather + segment-sum via PE matmul
with per-chunk sparse selection matrices), then an AllGather replicates
the level output for the next hop's gathers.  The dense GRU/readout phase
is node-local and runs entirely on-core.
"""

import sys
import types

sys.path.insert(0, "/opt/trn_rl_repo")

import numpy as np

N, F, P, E = 10000, 128, 4, 160000
C, HID = 512, 256
CORES = 8
M = N // CORES          # 1250 real nodes per core
MP = 1280               # padded nodes per core
NP = CORES * MP         # 10240 padded global nodes
TILES = MP // 128       # 10 dst tiles per core
GSZ = 6                 # segment-sum chunks (of 128 edges) per dma_gather
WCOLS = GSZ * 128 // 16  # int16 index columns per gather

TRACE = False           # set kernel.TRACE=True before calling for profiling
LAST_RESULT = {}        # exec_time_ns etc. for test harness introspection

_CACHE = {}


def _install_ntff_hook():
    """trace=True under axon needs antenv.axon_hooks, absent in this image."""
    import antenv

    if "antenv.axon_hooks" in sys.modules:
        return
    mod = types.ModuleType("antenv.axon_hooks")
    _h = [None]
    mod.set_axon_ntff_profile_hook = lambda h: _h.__setitem__(0, h)
    mod.get_axon_ntff_profile_hook = lambda: _h[0]
    sys.modules["antenv.axon_hooks"] = mod
    antenv.axon_hooks = mod
    try:
        from trn_agent_boot.trn_boot import _ntff_profile_via_ctypes

        mod.set_axon_ntff_profile_hook(
            _ntff_profile_via_ctypes("/opt/axon/libaxon_pjrt.so")
        )
    except Exception:
        pass


def _build_program(KCH, NG, need_bias, probs, b2v):
    import concourse.bacc as bacc
    import concourse.mybir as mybir
    from concourse import tile
    from concourse.masks import make_identity
    from contextlib import ExitStack

    f32 = mybir.dt.float32
    f32r = mybir.dt.float32r
    i16 = mybir.dt.int16
    AF = mybir.ActivationFunctionType
    OP = mybir.AluOpType

    nc = bacc.Bacc(
        "TRN2",
        target_bir_lowering=False,
        debug=False,
        enable_asserts=True,
        num_devices=CORES,
    )
    xpk = nc.dram_tensor("xpk", [NP, C], f32, kind="ExternalInput")
    sblk = nc.dram_tensor("sblk", [128, TILES * KCH * 128], f32, kind="ExternalInput")
    gidx = nc.dram_tensor("gidx", [128, TILES * NG * WCOLS], i16, kind="ExternalInput")
    wcat = nc.dram_tensor("wcat", [128, 4 * C], f32, kind="ExternalInput")
    lcat = nc.dram_tensor("lcat", [128, 12 * C], f32, kind="ExternalInput")
    lw1 = nc.dram_tensor("lw1", [128, 4 * HID], f32, kind="ExternalInput")
    lw2 = nc.dram_tensor("lw2", [128, 2], f32, kind="ExternalInput")
    biasin = nc.dram_tensor("biasin", [128, 16], f32, kind="ExternalInput")
    out_h = nc.dram_tensor("out_h", [MP, C], f32, kind="ExternalOutput")
    out_y = nc.dram_tensor("out_y", [MP, 1], f32, kind="ExternalOutput")

    with tile.TileContext(nc) as tc, ExitStack() as ctx:
        dram = ctx.enter_context(tc.tile_pool(name="dram", bufs=1, space="DRAM"))
        yl = [
            dram.tile([MP, C], f32, name=f"yl{L}", tag=f"yl{L}") for L in range(5)
        ]
        yf = [
            dram.tile([NP, C], f32, name=f"yf{L}", tag=f"yf{L}", addr_space="Shared")
            for L in range(4)
        ]
        p0 = ctx.enter_context(tc.tile_pool(name="p0", bufs=1))
        gidx_t = p0.tile([128, TILES * NG * WCOLS], i16)
        ident = p0.tile([128, 128], f32)
        identr = p0.tile([128, 128], f32r)
        nc.sync.dma_start(gidx_t[:], gidx[:, :])
        make_identity(nc, ident[:])
        nc.gpsimd.dma_start(identr[:], ident[:])

        nreg = nc.gpsimd.to_reg(GSZ * 128)

        # ---------------- message passing: Y_{k+1} = A Y_k ----------------
        with tc.tile_pool(name="msg", bufs=1) as pm, tc.tile_pool(
            name="mpsum", bufs=3, space="PSUM"
        ) as mpsum:
            s_t = pm.tile([128, TILES * KCH * 128], f32r, bufs=1)
            nc.gpsimd.dma_start(s_t[:], sblk[:, :])
            for L in range(5):
                # level 0 runs plain-fp32 matmuls on bitcast views so the
                # unrounded external input needs no f32r pre-pass
                src = xpk if L == 0 else yf[L - 1]
                for t in range(TILES):
                    yp = mpsum.tile([128, C], f32, tag="ypsum", name="yp")
                    for g in range(NG):
                        G = pm.tile(
                            [128, GSZ * C], f32r, tag="G", bufs=5, name="G"
                        )
                        cb = (t * NG + g) * WCOLS
                        if L == 0:
                            gout = G[:].bitcast(f32).rearrange(
                                "p (c e) -> p c e", e=C
                            )
                            gin = src[:, :]
                        else:
                            gout = G[:].rearrange("p (c e) -> p c e", e=C)
                            gin = src[:, :].bitcast(f32r)
                        nc.gpsimd.dma_gather(
                            out_ap=gout,
                            in_ap=gin,
                            idxs_ap=gidx_t[:, cb : cb + WCOLS],
                            num_idxs=GSZ * 128,
                            num_idxs_reg=nreg,
                            elem_size=C,
                        )
                        for k in range(GSZ):
                            ch = g * GSZ + k
                            scol = (t * KCH + ch) * 128
                            if L == 0:
                                lhs = s_t[:, scol : scol + 128].bitcast(f32)
                                rhs = G[:, k * C : (k + 1) * C].bitcast(f32)
                            else:
                                lhs = s_t[:, scol : scol + 128]
                                rhs = G[:, k * C : (k + 1) * C]
                            nc.tensor.matmul(
                                yp[:],
                                lhsT=lhs,
                                rhs=rhs,
                                start=(ch == 0),
                                stop=(ch == KCH - 1),
                            )
                    ysb = pm.tile([128, C], f32r, tag="ysb", bufs=3, name="ysb")
                    nc.vector.tensor_copy(ysb[:], yp[:])
                    nc.sync.dma_start(yl[L][t * 128 : (t + 1) * 128, :], ysb[:].bitcast(f32))
                if L < 4:
                    nc.gpsimd.collective_compute(
                        "AllGather",
                        mybir.AluOpType.bypass,
                        replica_groups=[list(range(CORES))],
                        ins=[yl[L][:, :].opt()],
                        outs=[yf[L][:, :].opt()],
                    )

        # ---------------- dense node-local phase ----------------
        with tc.tile_pool(name="dn", bufs=1) as pd, tc.tile_pool(
            name="dpsum", bufs=2, space="PSUM"
        ) as dpsum:
            wcat_t = pd.tile([128, 4 * C], f32r, bufs=1)
            lcat_t = pd.tile([128, 12 * C], f32r, bufs=1)
            lw1_t = pd.tile([128, 4 * HID], f32r, bufs=1)
            lw2_t = pd.tile([128, 2], f32r, bufs=1)
            bias_t = pd.tile([128, 16], f32, bufs=1)
            nc.gpsimd.dma_start(wcat_t[:], wcat[:, :])
            nc.gpsimd.dma_start(lcat_t[:], lcat[:, :])
            nc.gpsimd.dma_start(lw1_t[:], lw1[:, :])
            nc.gpsimd.dma_start(lw2_t[:], lw2[:, :])
            nc.sync.dma_start(bias_t[:], biasin[:, :])

            for ng0, nt in [(0, 512), (512, 512), (1024, 256)]:
                nb = nt // 128
                # transpose Y1/Y5 node-rows into feature-major tiles
                yT = {}
                for nm, ylsrc in [("y1", yl[0]), ("y5", yl[4])]:
                    for k4 in range(4):
                        yT[(nm, k4)] = pd.tile(
                            [128, nt], f32r, tag=f"{nm}T{k4}", bufs=1, name="yTt"
                        )
                    for b in range(nb):
                        rt = pd.tile([128, C], f32r, tag="rt", bufs=3, name="rt")
                        nc.sync.dma_start(
                            rt[:],
                            ylsrc[ng0 + b * 128 : ng0 + (b + 1) * 128, :].bitcast(f32r),
                        )
                        for k4 in range(4):
                            tp = dpsum.tile(
                                [128, 128], f32r, tag="tp", name="tp"
                            )
                            nc.tensor.transpose(
                                tp[:], rt[:, k4 * 128 : (k4 + 1) * 128], identr[:]
                            )
                            nc.vector.tensor_copy(
                                yT[(nm, k4)][:, b * 128 : (b + 1) * 128], tp[:]
                            )
                accs = [None] * 4
                for p in range(P):
                    H = [
                        pd.tile([128, nt], f32r, tag=f"H{m}", bufs=2, name="Ht_")
                        for m in range(4)
                    ]
                    for m in range(4):
                        hp = dpsum.tile([128, nt], f32, tag="dp", name="hp")
                        nc.tensor.matmul(
                            hp[:],
                            lhsT=wcat_t[:, m * 128 : (m + 1) * 128],
                            rhs=yT[("y5", p)][:],
                            start=True,
                            stop=True,
                        )
                        nc.vector.tensor_copy(H[m][:], hp[:])
                    gates = []
                    for gi, func in [(1, AF.Sigmoid), (2, AF.Sigmoid)]:
                        gt = [
                            pd.tile(
                                [128, nt], f32, tag=f"g{gi}{m}", bufs=1, name="gt"
                            )
                            for m in range(4)
                        ]
                        for m in range(4):
                            zp = dpsum.tile([128, nt], f32, tag="dp", name="zp")
                            nc.tensor.matmul(
                                zp[:],
                                lhsT=wcat_t[:, gi * C + m * 128 : gi * C + (m + 1) * 128],
                                rhs=yT[("y1", p)][:],
                                start=True,
                                stop=False,
                            )
                            Lsec = (gi - 1) * 4 * C
                            for k in range(4):
                                nc.tensor.matmul(
                                    zp[:],
                                    lhsT=lcat_t[
                                        :,
                                        Lsec + k * C + m * 128 : Lsec + k * C + (m + 1) * 128,
                                    ],
                                    rhs=H[k][:],
                                    start=False,
                                    stop=(k == 3),
                                )
                            bcol = (gi - 1) * 4 + m
                            if need_bias:
                                nc.scalar.activation(
                                    gt[m][:], zp[:], func,
                                    bias=bias_t[:, bcol : bcol + 1],
                                )
                            else:
                                nc.scalar.activation(gt[m][:], zp[:], func)
                        gates.append(gt)
                    Z, Rg = gates
                    HR = [
                        pd.tile([128, nt], f32r, tag=f"HR{m}", bufs=1, name="HRt")
                        for m in range(4)
                    ]
                    for m in range(4):
                        nc.vector.tensor_tensor(
                            out=HR[m][:], in0=H[m][:].bitcast(f32),
                            in1=Rg[m][:], op=OP.mult,
                        )
                    newacc = [None] * 4
                    for m in range(4):
                        tp2 = dpsum.tile([128, nt], f32, tag="dp", name="tp2")
                        nc.tensor.matmul(
                            tp2[:],
                            lhsT=wcat_t[:, 3 * C + m * 128 : 3 * C + (m + 1) * 128],
                            rhs=yT[("y1", p)][:],
                            start=True,
                            stop=False,
                        )
                        for k in range(4):
                            nc.tensor.matmul(
                                tp2[:],
                                lhsT=lcat_t[
                                    :,
                                    8 * C + k * C + m * 128 : 8 * C + k * C + (m + 1) * 128,
                                ],
                                rhs=HR[k][:],
                                start=False,
                                stop=(k == 3),
                            )
                        Htn = pd.tile([128, nt], f32, tag="Htn", bufs=2, name="Htn")
                        if need_bias:
                            nc.scalar.activation(
                                Htn[:], tp2[:], AF.Tanh, bias=bias_t[:, 8 + m : 9 + m]
                            )
                        else:
                            nc.scalar.activation(Htn[:], tp2[:], AF.Tanh)
                        d1 = pd.tile([128, nt], f32, tag="d1", bufs=2, name="d1")
                        nc.vector.tensor_tensor(
                            out=d1[:], in0=H[m][:].bitcast(f32), in1=Htn[:],
                            op=OP.subtract,
                        )
                        d2 = pd.tile([128, nt], f32, tag="d2", bufs=2, name="d2")
                        nc.vector.tensor_tensor(
                            out=d2[:], in0=Z[m][:], in1=d1[:], op=OP.mult
                        )
                        hn = pd.tile([128, nt], f32, tag="hn", bufs=2, name="hn")
                        nc.vector.tensor_tensor(
                            out=hn[:], in0=d2[:], in1=Htn[:], op=OP.add
                        )
                        na = pd.tile([128, nt], f32, tag=f"acc{m}", bufs=2, name="na")
                        if p == 0:
                            nc.vector.tensor_scalar_mul(na[:], hn[:], float(probs[0]))
                        else:
                            nc.vector.scalar_tensor_tensor(
                                out=na[:], in0=hn[:], scalar=float(probs[p]),
                                in1=accs[m][:], op0=OP.mult, op1=OP.add,
                            )
                        newacc[m] = na
                    accs = newacc
                # readout
                rl = [
                    pd.tile([128, nt], f32r, tag=f"rl{m}", bufs=1, name="rl")
                    for m in range(4)
                ]
                for m in range(4):
                    nc.scalar.activation(rl[m][:], accs[m][:], AF.Relu)
                h1 = []
                for hm in range(2):
                    pp = dpsum.tile([128, nt], f32, tag="dp", name="pp")
                    for k in range(4):
                        nc.tensor.matmul(
                            pp[:],
                            lhsT=lw1_t[:, k * HID + hm * 128 : k * HID + (hm + 1) * 128],
                            rhs=rl[k][:],
                            start=(k == 0),
                            stop=(k == 3),
                        )
                    t1 = pd.tile([128, nt], f32r, tag=f"h1{hm}", bufs=1, name="t1")
                    if need_bias:
                        nc.scalar.activation(
                            t1[:], pp[:], AF.Relu, bias=bias_t[:, 12 + hm : 13 + hm]
                        )
                    else:
                        nc.scalar.activation(t1[:], pp[:], AF.Relu)
                    h1.append(t1)
                py = dpsum.tile([1, nt], f32, tag="py", name="py")
                for hm in range(2):
                    nc.tensor.matmul(
                        py[:],
                        lhsT=lw2_t[:, hm : hm + 1],
                        rhs=h1[hm][:],
                        start=(hm == 0),
                        stop=(hm == 1),
                    )
                oy = pd.tile([1, nt], f32, tag="oy", bufs=2, name="oy")
                nc.vector.tensor_scalar_add(oy[:], py[:], float(b2v))
                nc.sync.dma_start(
                    out_y[ng0 : ng0 + nt, 0:1].rearrange("a b -> b a"),
                    oy[0:1, :nt],
                )
                for m in range(4):
                    for b in range(nb):
                        tp = dpsum.tile([128, 128], f32, tag="tp", name="tp3")
                        nc.tensor.transpose(
                            tp[:], accs[m][:, b * 128 : (b + 1) * 128], ident[:]
                        )
                        hsb = pd.tile([128, 128], f32, tag="hsb", bufs=3, name="hsb")
                        nc.vector.tensor_copy(hsb[:], tp[:])
                        nc.sync.dma_start(
                            out_h[
                                ng0 + b * 128 : ng0 + (b + 1) * 128,
                                m * 128 : (m + 1) * 128,
                            ],
                            hsb[:],
                        )
    nc.compile()
    return nc


def kernel(**inputs):
    x = np.asarray(inputs["x"], np.float32)
    edge_index = np.asarray(inputs["edge_index"])
    edge_attr = np.asarray(inputs["edge_attr"], np.float32)

    # ---- graph preprocessing (host): norm, partition, dst-sorted chunks ----
    src = np.concatenate([edge_index[0], np.arange(N)]).astype(np.int64)
    dst = np.concatenate([edge_index[1], np.arange(N)]).astype(np.int64)
    ew = np.concatenate([edge_attr, np.ones(N, np.float32)]).astype(np.float32)
    deg = np.zeros(N, np.float32)
    np.add.at(deg, dst, ew)
    dinv = np.where(deg > 0, 1.0 / np.sqrt(np.where(deg > 0, deg, 1.0)), 0.0).astype(
        np.float32
    )
    norm = (dinv[src] * ew * dinv[dst]).astype(np.float32)
    core_of = dst // M
    dst_local = dst % M
    src_pad = ((src // M) * MP + (src % M)).astype(np.int64)

    # per (core, tile) edge lists
    per_ct = [[None] * TILES for _ in range(CORES)]
    counts = np.zeros((CORES, TILES), np.int64)
    tile_of = dst_local // 128
    key = core_of * TILES + tile_of
    order = np.argsort(key * (M + 1) + dst_local, kind="stable")
    s_src, s_norm, s_dl, s_key = (
        src_pad[order],
        norm[order],
        dst_local[order],
        key[order],
    )
    bounds = np.searchsorted(s_key, np.arange(CORES * TILES + 1))
    for c in range(CORES):
        for t in range(TILES):
            a, b = bounds[c * TILES + t], bounds[c * TILES + t + 1]
            per_ct[c][t] = (s_src[a:b], s_norm[a:b], s_dl[a:b] - t * 128)
            counts[c, t] = b - a
    KCH_need = int(np.max((counts + 127) // 128))
    NG = (KCH_need + GSZ - 1) // GSZ
    KCH = NG * GSZ

    sblks = []
    gidxs = []
    for c in range(CORES):
        S = np.zeros((128, TILES * KCH, 128), np.float32)
        IDX = np.zeros((TILES * KCH * 128,), np.int16)
        for t in range(TILES):
            e_src, e_norm, e_d = per_ct[c][t]
            n = len(e_src)
            base = t * KCH * 128
            sl = np.arange(n)
            S[sl % 128, t * KCH + sl // 128, e_d] = e_norm
            IDX[base : base + n] = e_src.astype(np.int16)
        # wrap indices per gather: [16, WCOLS] blocks replicated to 128 parts
        gi = np.zeros((128, TILES * NG * WCOLS), np.int16)
        for t in range(TILES):
            for g in range(NG):
                a = t * KCH * 128 + g * GSZ * 128
                arr = IDX[a : a + GSZ * 128]
                blk = arr.reshape(WCOLS, 16).T  # [16, WCOLS]
                gi[:, (t * NG + g) * WCOLS : (t * NG + g + 1) * WCOLS] = np.tile(
                    blk, (8, 1)
                )
        sblks.append(S.reshape(128, TILES * KCH * 128))
        gidxs.append(gi)

    # ---- weight composition (host, fp64 -> fp32) ----
    W = [np.asarray(inputs[f"W{i}"], np.float64) for i in range(1, 6)]
    bvec = [np.asarray(inputs[f"b{i}"], np.float64) for i in range(1, 6)]
    Lz = np.asarray(inputs["Lz_w"], np.float64)
    Lr = np.asarray(inputs["Lr_w"], np.float64)
    Lh = np.asarray(inputs["Lh_w"], np.float64)
    Wc = W[0]
    for Wi in W[1:]:
        Wc = Wc @ Wi
    Wzp = np.asarray(inputs["Wz"], np.float64) @ Lz[:C]
    Wrp = np.asarray(inputs["Wr"], np.float64) @ Lr[:C]
    Whp = np.asarray(inputs["Wh"], np.float64) @ Lh[:C]
    wcat = np.concatenate([Wc, Wzp, Wrp, Whp], axis=1).astype(np.float32)

    def chunkrows(A):  # [512, X] -> [128, 4*X]
        return np.concatenate([A[k * 128 : (k + 1) * 128] for k in range(4)], 1)

    lcat = np.concatenate(
        [
            chunkrows(Lz[C:].astype(np.float32)),
            chunkrows(Lr[C:].astype(np.float32)),
            chunkrows(Lh[C:].astype(np.float32)),
        ],
        axis=1,
    )
    lw1 = chunkrows(np.asarray(inputs["lin1_w"], np.float32))
    lw2m = np.asarray(inputs["lin2_w"], np.float32)  # [256, 1]
    lw2 = np.stack([lw2m[:128, 0], lw2m[128:, 0]], axis=1)  # [128, 2]

    bz = np.asarray(inputs["bz"], np.float64) @ Lz[:C] + np.asarray(
        inputs["Lz_b"], np.float64
    )
    br = np.asarray(inputs["br"], np.float64) @ Lr[:C] + np.asarray(
        inputs["Lr_b"], np.float64
    )
    bh = np.asarray(inputs["bh"], np.float64) @ Lh[:C] + np.asarray(
        inputs["Lh_b"], np.float64
    )
    lin1_b = np.asarray(inputs["lin1_b"], np.float64)
    biasarr = np.zeros((128, 16), np.float32)
    for mm in range(4):
        biasarr[:, mm] = bz[mm * 128 : (mm + 1) * 128]
        biasarr[:, 4 + mm] = br[mm * 128 : (mm + 1) * 128]
        biasarr[:, 8 + mm] = bh[mm * 128 : (mm + 1) * 128]
    biasarr[:, 12] = lin1_b[:128]
    biasarr[:, 13] = lin1_b[128:]
    need_bias = bool(np.abs(biasarr).max() > 0)
    # stacked-conv bias correction must be zero for the composed-weight path
    assert all(np.abs(b).max() == 0 for b in bvec), "nonzero conv bias unsupported"

    attn = np.asarray(inputs["attn"], np.float64)
    probs = np.exp(attn - attn.max())
    probs = (probs / probs.sum()).astype(np.float32)
    b2v = float(np.asarray(inputs["lin2_b"])[0])

    # ---- packed node features, padded layout ----
    xpk = np.zeros((NP, C), np.float32)
    xr = x.transpose(0, 2, 1).reshape(N, P * F)
    for c in range(CORES):
        xpk[c * MP : c * MP + M] = xr[c * M : (c + 1) * M]

    # ---- build / fetch program ----
    key2 = (KCH, NG, need_bias, tuple(np.round(probs, 7)), round(b2v, 7))
    if key2 not in _CACHE:
        _CACHE[key2] = _build_program(KCH, NG, need_bias, probs, b2v)
    nc = _CACHE[key2]

    from concourse.bass_utils import run_bass_kernel_spmd

    in_maps = []
    for c in range(CORES):
        in_maps.append(
            {
                "xpk": xpk,
                "sblk": sblks[c],
                "gidx": gidxs[c],
                "wcat": wcat,
                "lcat": lcat,
                "lw1": lw1,
                "lw2": lw2,
                "biasin": biasarr,
            }
        )
    if TRACE:
        _install_ntff_hook()
    res = run_bass_kernel_spmd(
        nc, in_maps, core_ids=list(range(CORES)), trace=TRACE
    )
    LAST_RESULT["exec_time_ns"] = res.exec_time_ns
    LAST_RESULT["res"] = res

    out_hidden = np.concatenate(
        [res.results[c]["out_h"][:M] for c in range(CORES)], axis=0
    )
    out = np.concatenate([res.results[c]["out_y"][:M] for c in range(CORES)], axis=0)
    return out, out_hidden
